# revision 42
# baseline (speedup 1.0000x reference)
"""Causal self-attention (B=4, S=2048, D=1024, H=16) on 8 TRN2 NeuronCores.

Sharding: core c -> batch b = c//2, head-group g = c%2 (8 heads each).
Per core: QKV projection for its 8 heads, causal attention, partial out-proj
over its 512 y-features. Host sums the two partials per batch and adds b_out.

On-device layout ("T" = feature-major so matmul contraction sits on SBUF
partitions; all matmul operands are float32r = full-rate fp32):
  xT      [128, 8, 2048]  xT[p,t,s] = x[b][s, t*128+p]
  wqkvT   [128, 8, 1536]  wqkvT[p,t,f] = w_shard[f, t*128+p]
  qk      [128, 8, 512]   per s-block q/k in T layout (f-tiles 0-3 q, 4-7 k)
  v'      [128, 4, 520]   per s-block v natural + per-head ones column (denom)
  attT    [sk 128, sq]    scores transposed; softmax without max-subtraction
  yT      [128, 4, 512]   per s-block; out_part [8, 128, 2048] (host transposes)
"""

import numpy as np

import concourse.bacc as bacc
import concourse.mybir as mybir
from concourse.tile import TileContext
from concourse.bass_utils import run_bass_kernel_spmd

F32R = mybir.dt.float32r
F32 = mybir.dt.float32
AF = mybir.ActivationFunctionType
OP = mybir.AluOpType

B, S, D = 4, 2048, 1024
H = 16
HD = 64
HL = 8          # heads per core
SB = 512        # sequence block
NJ = S // SB    # 4 s-blocks
DT = D // 128   # 8 contraction tiles

_CACHED_NC = None


def build_nc():
    nc = bacc.Bacc(None, target_bir_lowering=False)

    xT = nc.dram_tensor("xT", [128, DT, S], F32R, kind="ExternalInput")
    # weight layouts are tile-contiguous per partition for efficient DMA
    wqk = nc.dram_tensor("wqk", [128, 8, DT, 128], F32R, kind="ExternalInput")
    wvd = nc.dram_tensor("wvd", [128, DT, 512], F32R, kind="ExternalInput")
    wod = nc.dram_tensor("wod", [128, 8, 4, 128], F32R, kind="ExternalInput")
    bqkv = nc.dram_tensor("bqkv", [128, 12], F32, kind="ExternalInput")
    bv = nc.dram_tensor("bv", [1, 512], F32, kind="ExternalInput")
    tri = nc.dram_tensor("tri", [128, 128], F32R, kind="ExternalInput")
    out = nc.dram_tensor("out_part", [8, 128, S], F32, kind="ExternalOutput")

    with TileContext(nc) as tc:
        with (
            tc.tile_pool(name="const", bufs=1) as cpool,
            tc.tile_pool(name="big", bufs=1) as bpool,
            tc.tile_pool(name="qk", bufs=4) as qkpool,
            tc.tile_pool(name="vps", bufs=4) as vpool,
            tc.tile_pool(name="wq", bufs=3) as wqpool,
            tc.tile_pool(name="wo", bufs=2) as wopool,
            tc.tile_pool(name="xb", bufs=2) as xpool,
            tc.tile_pool(name="expt", bufs=4) as epool,
            tc.tile_pool(name="yt", bufs=2) as ypool,
            tc.tile_pool(name="ob", bufs=2) as opool,
            tc.tile_pool(name="rb", bufs=2) as rpool,
            tc.tile_pool(name="p1", bufs=2, space="PSUM") as p1,
            tc.tile_pool(name="pscore", bufs=3, space="PSUM") as pscore,
            tc.tile_pool(name="py", bufs=3, space="PSUM") as py,
        ):
            # ---- constants (tiles now; DMAs emitted after the first x/w
            # loads so the hot path owns the DMA queues at startup) ----
            tri_t = cpool.tile([128, 128], F32R)
            bqkv_t = cpool.tile([128, 12], F32)
            bv_t = cpool.tile([1, 512], F32)
            bvb = cpool.tile([128, 512], F32)

            ones_f32 = cpool.tile([128, 128], F32)
            nc.vector.memset(ones_f32[:], 1.0)
            zf32 = cpool.tile([128, 384], F32)
            nc.vector.memset(zf32[:], 0.0)
            zeros = cpool.tile([128, 384], F32R)
            nc.vector.tensor_copy(zeros[:], zf32[:])

            wv = bpool.tile([128, DT, 512], F32R)

            def load_consts():
                nc.sync.dma_start(wv[:], wvd[:])
                nc.sync.dma_start(tri_t[:], tri[:])
                nc.sync.dma_start(bv_t[:], bv[:])
                nc.gpsimd.partition_broadcast(bvb[:], bv_t[:])

            src_view = ones_f32[:, :32].rearrange("p (a h) -> p a h", h=HL)

            qk_blk = []
            vp_blk = []

            # ---- stage 1 pieces (emitted interleaved with attention) ----
            def stage1_qk(j, f_lo, f_hi):
                xb, qk = xblk[j]
                for f in range(f_lo, f_hi):
                    w = wqpool.tile([128, DT, 128], F32R)
                    nc.sync.dma_start(w[:], wqk[:, f])
                    ps = p1.tile([128, SB], F32, tag="ps")
                    for d in range(DT):
                        nc.tensor.matmul(
                            ps[:], w[:, d, :], xb[:, d, :],
                            start=(d == 0), stop=(d == DT - 1),
                        )
                    nc.vector.tensor_scalar(
                        qk[:, f, :], ps[:], bqkv_t[:, f:f + 1], None, OP.add
                    )

            def stage1_v(j):
                xb, _ = xblk[j]
                vp = vpool.tile([128, 4, HL * 65], F32R)
                vp_blk.append(vp)
                ones_view = vp[:].rearrange(
                    "p a (h c) -> p a h c", c=65)[:, :, :, 64]
                nc.vector.tensor_copy(ones_view, src_view)
                for s4 in range(4):
                    ps = p1.tile([128, 512], F32, tag="ps")
                    for d in range(DT):
                        nc.tensor.matmul(
                            ps[:], xb[:, d, s4 * 128:(s4 + 1) * 128], wv[:, d, :],
                            start=(d == 0), stop=(d == DT - 1),
                        )
                    for h in range(HL):
                        nc.vector.tensor_tensor(
                            vp[:, s4, h * 65:h * 65 + 64],
                            ps[:, h * 64:(h + 1) * 64],
                            bvb[:, h * 64:(h + 1) * 64],
                            OP.add,
                        )

            xblk = {}

            def stage1_load(j):
                xb = xpool.tile([128, DT, SB], F32R)
                for d in range(DT):  # split so first matmul starts early
                    nc.sync.dma_start(xb[:, d, :], xT[:, d, j * SB:(j + 1) * SB])
                qk = qkpool.tile([128, 8, SB], F32R)
                qk_blk.append(qk)
                xblk[j] = (xb, qk)

            def attn_pair(j, m, yT):
                """Heads h0=2m (partitions 0-63) and h1=2m+1 (64-127).

                Score matmuls for the two heads land on different PE row
                groups (base_partition 0 vs 64 -> tile_position row 0/64)
                and execute concurrently in the array.
                """
                qk = qk_blk[j]
                ft = m
                n_sk = 4 * (j + 1)
                ps_y = [py.tile([128, SB], F32, tag="py", name=f"psy{half}") for half in range(2)]
                for i in range(n_sk):
                    ib, il = i // 4, i % 4
                    c0 = max(0, i * 128 - j * SB)
                    # matmul start col: keep N >= 256 so fp32r streams at
                    # 1 cycle/row; the [s0, c0) strip is exp'd or zeroed
                    s0 = min(c0, SB - 256)
                    es = []
                    for half in range(2):
                        b0 = half * 64
                        ks = qk_blk[ib][b0:b0 + 64, 4 + ft,
                                        il * 128:il * 128 + 128]
                        qs = qk[b0:b0 + 64, ft, :]
                        ps_s = pscore.tile([128, SB], F32)
                        nc.tensor.matmul(
                            ps_s[:, s0:], ks, qs[:, s0:], start=True, stop=True
                        )
                        e = epool.tile([128, SB], F32R)
                        if s0 < c0:
                            nc.gpsimd.tensor_copy(
                                e[:, s0:c0], zeros[:, :c0 - s0]
                            )
                        nc.scalar.activation(
                            e[:, c0:], ps_s[:, c0:], AF.Exp, scale=0.125
                        )
                        if i >= 4 * j:
                            nc.gpsimd.tensor_tensor(
                                e[:, c0:c0 + 128], e[:, c0:c0 + 128], tri_t[:],
                                OP.mult,
                            )
                        es.append(e)
                    for half in range(2):
                        h = 2 * m + half
                        nc.tensor.matmul(
                            ps_y[half][:65, s0:],
                            vp_blk[ib][:, il, h * 65:h * 65 + 65],
                            es[half][:, s0:],
                            start=(i == 0), stop=(i == n_sk - 1),
                        )
                for half in range(2):
                    h = 2 * m + half
                    recip = rpool.tile([1, SB], F32, tag="recip")
                    nc.vector.reciprocal(recip[:], ps_y[half][64:65, :])
                    rb = rpool.tile([64, SB], F32, tag="rb")
                    nc.gpsimd.partition_broadcast(rb[:], recip[:])
                    nc.vector.tensor_tensor(
                        yT[half * 64:half * 64 + 64, ft, :],
                        ps_y[half][:64, :], rb[:], OP.mult,
                    )

            def outproj(j, yT, t_lo, t_hi):
                for t in range(t_lo, t_hi):
                    wo = wopool.tile([128, 4, 128], F32R)
                    nc.sync.dma_start(wo[:], wod[:, t])
                    ps = p1.tile([128, SB], F32, tag="ps")
                    for ff in range(4):
                        nc.tensor.matmul(
                            ps[:], wo[:, ff, :], yT[:, ff, :],
                            start=(ff == 0), stop=(ff == 3),
                        )
                    ob = opool.tile([128, SB], F32)
                    nc.vector.tensor_copy(ob[:], ps[:])
                    nc.sync.dma_start(out[t, :, j * SB:(j + 1) * SB], ob[:])

            # ---- main pipeline ----
            nc.sync.dma_start(bqkv_t[:], bqkv[:])
            stage1_load(0)
            stage1_qk(0, 0, 8)
            load_consts()
            stage1_v(0)
            yts = {}
            for j in range(NJ):
                yT = ypool.tile([128, 4, SB], F32R)
                yts[j] = yT
                for m in range(4):
                    attn_pair(j, m, yT)
                    if j + 1 < NJ:
                        if m == 0:
                            stage1_load(j + 1)
                            stage1_qk(j + 1, 0, 2)
                        elif m == 1:
                            stage1_qk(j + 1, 2, 4)
                        elif m == 2:
                            stage1_qk(j + 1, 4, 6)
                        elif m == 3:
                            stage1_qk(j + 1, 6, 8)
                            stage1_v(j + 1)
                    elif m >= 1:  # j == NJ-1: fill ACT-bound tail with
                        outproj(NJ - 2, yts[NJ - 2], 3 * (m - 1), min(3 * m, 8))
                if j < NJ - 2:
                    outproj(j, yT, 0, 8)
            outproj(NJ - 1, yts[NJ - 1], 0, 8)

    nc.finalize()
    return nc


def _prep_core_inputs(x, w_qkv, b_qkv, w_out, core):
    b = core // 2
    g = core % 2
    rows = np.concatenate([
        w_qkv[512 * g:512 * g + 512],
        w_qkv[1024 + 512 * g:1024 + 512 * g + 512],
        w_qkv[2048 + 512 * g:2048 + 512 * g + 512],
    ], axis=0)  # [1536, 1024]
    brows = np.concatenate([
        b_qkv[512 * g:512 * g + 512],
        b_qkv[1024 + 512 * g:1024 + 512 * g + 512],
        b_qkv[2048 + 512 * g:2048 + 512 * g + 512],
    ])  # [1536]

    xT = np.ascontiguousarray(
        x[b].T.reshape(DT, 128, S).transpose(1, 0, 2)
    ).astype(np.float32)
    # rows.T[d, f]: [1024, 1536] -> wqk [p, ftile(8), d(8), fc(128)]
    rT = rows.T.reshape(DT, 128, 12, 128)          # [d, p, ftile, fc]
    wqk = np.ascontiguousarray(
        rT[:, :, :8].transpose(1, 2, 0, 3)).astype(np.float32)
    wvd = np.ascontiguousarray(
        rows.T.reshape(DT, 128, 1536)[:, :, 1024:].transpose(1, 0, 2)
    ).astype(np.float32)
    ws = w_out[:, 512 * g:512 * g + 512]           # [do(1024), f(512)]
    wod = np.ascontiguousarray(
        ws.reshape(8, 128, 4, 128).transpose(3, 0, 2, 1)).astype(np.float32)
    bqkv = np.ascontiguousarray(brows[:1536].reshape(12, 128).T).astype(np.float32)
    bv = np.ascontiguousarray(brows[1024:1536].reshape(1, 512)).astype(np.float32)
    tri = (np.arange(128)[:, None] <= np.arange(128)[None, :]).astype(np.float32)

    return {
        "xT": xT, "wqk": wqk, "wvd": wvd, "wod": wod,
        "bqkv": bqkv, "bv": bv, "tri": tri,
    }


def kernel(x, w_qkv, b_qkv, w_out, b_out):
    global _CACHED_NC
    x = np.asarray(x, dtype=np.float32)
    w_qkv = np.asarray(w_qkv, dtype=np.float32)
    b_qkv = np.asarray(b_qkv, dtype=np.float32)
    w_out = np.asarray(w_out, dtype=np.float32)
    b_out = np.asarray(b_out, dtype=np.float32)

    if _CACHED_NC is None:
        _CACHED_NC = build_nc()
    nc = _CACHED_NC

    in_maps = [
        _prep_core_inputs(x, w_qkv, b_qkv, w_out, c) for c in range(8)
    ]
    last_err = None
    for attempt in range(5):
        try:
            res = run_bass_kernel_spmd(nc, in_maps, core_ids=list(range(8)))
            break
        except Exception as e:  # transient NRT/axon wedge: retry
            last_err = e
            import time
            time.sleep(20)
    else:
        raise last_err

    out = np.empty((B, S, D), dtype=np.float32)
    for b in range(B):
        p0 = res.results[2 * b]["out_part"]
        p1_ = res.results[2 * b + 1]["out_part"]
        tot = (p0 + p1_).reshape(D, S)  # [do, s]
        out[b] = tot.T + b_out[None, :]
    return out
